# revision 1
# baseline (speedup 1.0000x reference)
"""Trainium2 Bass kernel for nn_ButterflyLayer2D (butterfly 2D CNN).

Strategy: pure data parallel over 8 NeuronCores (16 batch each), with the
per-core batch processed in 2 phases of 8 to fit SBUF.

All tensors are pre-arranged on the host (numpy) into DMA-friendly layouts:
  - activations live in SBUF as [128 = (w%2)*64 + c, (node, b, h, w//2)]
    so each 2x2-stride-2 per-node conv becomes 4 fp32r matmuls with K=128
    chunks: col-group q = output w-parity (tile_position (0, 64q)), x-chunks
    accumulate in PSUM. PSUM [128=(q,c_out), N] is evicted full-width by a
    single relu+bias op (alternating ScalarE/VectorE) directly into the next
    level's interleaved layout — zero data reshuffling anywhere on chip.
  - the input 4x4-patch conv uses the same trick with K=16 row-groups spread
    over 4 partition groups (one per b%4) for PE concurrency.
  - the final per-node dense is a [64,128] x [64,b] matmul; outputs are
    written as [128=(r,ou,ov), (ph,node,b)] and decoded on the host.
Weights are streamed from HBM in 8-node chunks through a recycled tile tag.
"""

import numpy as np
from contextlib import ExitStack

import concourse.bass as bass
import concourse.tile as tile
from concourse import bacc, mybir
from concourse.bass_utils import run_bass_kernel_spmd

F32 = mybir.dt.float32
F32R = mybir.dt.float32r
BF16 = mybir.dt.bfloat16
AF = mybir.ActivationFunctionType
ALU = mybir.AluOpType

B, IN, NLVL, KLVL, C = 128, 256, 6, 3, 64
TCOL = 1024               # psum tile columns
PBUFS = 4                 # psum tile bufs
NK, OU, OV = 8, 8, 8
NCORES = 8
BC = B // NCORES          # 16 per-core batch
PH = 1                    # phases per core
BG = BC // PH             # batch per phase
HALF = BG // 4            # input-conv b-subgroups per partition group
LVL_NODES = [4, 16, 64, 64, 64, 64]          # nodes per level
LVL_HIN = [64, 32, 16, 8, 4, 2]              # spatial H into each level
WGRP = 8                  # weight streaming chunk (nodes)


# ----------------------------------------------------------------------------
# host-side pre-arrangement
# ----------------------------------------------------------------------------

def _prep_weights(inputs):
    """Weights/biases blobs shared by all cores."""
    out = {}
    # input filter: lhsT [16=(p,q), 64], replicated at partition bases 0/32/64/96
    import ml_dtypes
    fin = inputs["in_filter"][:, :, 0, :].reshape(16, C).astype(np.float32)
    finr = np.zeros((128, C), np.float32)
    for g in range(4):
        finr[g * 32 : g * 32 + 16] = fin
    out["fin"] = finr.astype(ml_dtypes.bfloat16)
    out["bin"] = np.concatenate([inputs["in_bias"], inputs["in_bias"]]).reshape(
        128, 1
    ).astype(np.float32)

    for lvl in range(1, NLVL + 1):
        f = inputs[f"f{lvl}"].astype(np.float32)  # [n,n,2,2,C,C] (x,y,ci,co)
        n = f.shape[0]
        assert n == 2 ** min(lvl, KLVL)
        # lhsT per node: [(y*64+ci), (x*64+co)]
        w = f.transpose(0, 1, 3, 4, 2, 5).reshape(n * n, 2 * C, 2 * C)
        if lvl <= KLVL:
            # sibling-pair blob: per pair (u,2t)+(u,2t+1):
            # [(y,ci)=128, (x, coA|coB)=256] -> [128, pairs*256]
            wp = w.reshape(n * n // 2, 2, 2 * C, 2, C)  # [pair, s, (y,ci), x, co]
            wp = wp.transpose(2, 0, 3, 1, 4)            # [(y,ci), pair, x, s, co]
            out[f"w{lvl}"] = np.ascontiguousarray(wp).reshape(
                128, n * n * 128
            ).astype(ml_dtypes.bfloat16)
        else:
            # blob [128, nodes*128], free = (node, x*64+co)
            out[f"w{lvl}"] = np.ascontiguousarray(w.transpose(1, 0, 2)).reshape(
                128, n * n * 128
            ).astype(ml_dtypes.bfloat16)
        b = inputs[f"b{lvl}"].astype(np.float32).reshape(n * n, C)
        if lvl < NLVL:
            # [128, nodes]: rows (q,c) with bias duplicated across q
            bb = np.concatenate([b, b], axis=1)  # [nodes, 128]
            out[f"b{lvl}"] = np.ascontiguousarray(bb.T)
        else:
            # lvl6 node-pair scheme: psum rows = (cA, cB) for pair (2k, 2k+1)
            bb = b.reshape(n * n // 2, 2 * C)  # [pairs, (cA,cB)]
            out[f"b{lvl}"] = np.ascontiguousarray(bb.T)  # [128, 32]
    # dense: lhsT per node [64=c, 128=(r, ou*8+ov)]
    wd = inputs["Wd"].astype(np.float32).reshape(NK * NK, 2, C, OU * OV)
    wd = wd.transpose(2, 0, 1, 3).reshape(C, NK * NK * 2 * OU * OV)
    out["wd"] = np.ascontiguousarray(wd).astype(ml_dtypes.bfloat16)
    return out


def _prep_input(in_data_core):
    """Per-core input blob: [64 = (b%4)*16 + (i%4)*4 + (j%4),
    (ph, b//4%2, x=i//4, y4=j//4)] packed (no zero rows)."""
    ind = in_data_core[:, :, :, 0]  # [16, 256, 256]
    a = ind.reshape(PH, HALF, 4, 64, 4, 64, 4)  # [ph, half, g, x, p, y4, q]
    a = a.transpose(2, 4, 6, 0, 1, 3, 5)        # [g, p, q, ph, half, x, y4]
    import ml_dtypes
    return np.ascontiguousarray(a).reshape(64, PH * HALF * 64 * 64).astype(ml_dtypes.bfloat16)


def _decode_output(t2_core):
    """t2 [128=(r,ou,ov), (ph, node, bl)] -> [16, 64, 64, 2]."""
    t = t2_core.reshape(2, OU, OV, PH, NK, NK, BG)  # r,ou,ov,ph,u,v,bl
    t = t.transpose(3, 6, 4, 1, 5, 2, 0)            # ph,bl,u,ou,v,ov,r
    return np.ascontiguousarray(t).reshape(BC, NK * OU, NK * OV, 2)


# ----------------------------------------------------------------------------
# device kernel
# ----------------------------------------------------------------------------

def _build_kernel(reps=1, xouter=True):
    nc = bacc.Bacc(None, target_bir_lowering=False)
    p = {}
    p["a0"] = nc.declare_dram_parameter("a0", [64, PH * HALF * 64 * 64], BF16, isOutput=False)
    p["fin"] = nc.declare_dram_parameter("fin", [128, C], BF16, isOutput=False)
    p["bin"] = nc.declare_dram_parameter("bin", [128, 1], F32, isOutput=False)
    for lvl in range(1, NLVL + 1):
        n2 = LVL_NODES[lvl - 1]
        p[f"w{lvl}"] = nc.declare_dram_parameter(f"w{lvl}", [128, n2 * 128], BF16, isOutput=False)
        bcols = n2 if lvl < NLVL else n2 // 2
        p[f"b{lvl}"] = nc.declare_dram_parameter(f"b{lvl}", [128, bcols], F32, isOutput=False)
    p["wd"] = nc.declare_dram_parameter("wd", [64, NK * NK * 128], BF16, isOutput=False)
    t2 = nc.declare_dram_parameter("t2", [128, PH * NK * NK * BG], F32, isOutput=True)

    evict_ctr = [0]

    def evict(out_ap, psum_ap, bias_ap):
        """relu(psum + bias) -> sbuf, alternating engines to split the load."""
        evict_ctr[0] += 1
        if evict_ctr[0] % 2 == 0:
            nc.scalar.activation(out_ap, psum_ap, AF.Relu, bias=bias_ap)
        else:
            nc.vector.tensor_scalar(out_ap, psum_ap, bias_ap, 0.0,
                                    op0=ALU.add, op1=ALU.max)

    with tile.TileContext(nc) as tc, ExitStack() as ctx:
        const = ctx.enter_context(tc.tile_pool(name="const", bufs=1))
        wpool = ctx.enter_context(tc.tile_pool(name="wts", bufs=4))
        apool = ctx.enter_context(tc.tile_pool(name="acts", bufs=1))
        inpool = ctx.enter_context(tc.tile_pool(name="inp", bufs=1))
        fpool = ctx.enter_context(tc.tile_pool(name="feat", bufs=2))
        ppool = ctx.enter_context(tc.tile_pool(name="ps", bufs=PBUFS, space="PSUM"))
        spool = ppool

        # constants: input filter, biases (all small, loaded once)
        fin_t = const.tile([128, C], BF16)
        nc.sync.dma_start(fin_t[:], p["fin"][:])
        bin_t = const.tile([128, 1], F32)
        nc.sync.dma_start(bin_t[:], p["bin"][:])
        bias_t = {}
        for lvl in range(1, NLVL + 1):
            bcols = LVL_NODES[lvl - 1] if lvl < NLVL else LVL_NODES[lvl - 1] // 2
            bias_t[lvl] = const.tile([128, bcols], F32, tag=f"bias{lvl}", name=f"bias{lvl}")
            nc.sync.dma_start(bias_t[lvl][:], p[f"b{lvl}"][:])

        for phx in range(reps * PH):
            ph = phx % PH
            # ---------------- input staging ----------------
            a0s = inpool.tile([128, HALF * 64 * 64], BF16, tag="a0s", name=f"a0s{phx}")
            for g in range(4):
                nc.sync.dma_start(
                    a0s[g * 32 : g * 32 + 16, :],
                    p["a0"][g * 16 : (g + 1) * 16,
                            ph * HALF * 64 * 64 : (ph + 1) * HALF * 64 * 64],
                )
            a0v = a0s[:].rearrange("p (h x y) -> p h x y", h=HALF, x=64)

            # ---------------- input conv ----------------
            # X slab: [128=(y%2,c), (bl, x, y2)]  (bl=8, x=64, y2=32)
            X = apool.tile([128, BG * 64 * 32], BF16, tag="s0", name=f"x{phx}")
            Xv = X[:].rearrange("p (b h w) -> p b h w", b=BG, h=64)
            for bl in range(BG):
                g, half = bl % 4, bl // 4
                for xh in range(2048 // TCOL):
                    pt = ppool.tile([128, TCOL], F32, tag="ps",
                                    padded_shape=[128, TCOL],
                                    name=f"pin{phx}_{bl}_{xh}")
                    for sub in range(TCOL // 512):
                        xq = xh * (TCOL // 512) + sub
                        for q in (0, 1):
                            rhs = a0v[g * 32 : g * 32 + 16, half,
                                      xq * 16 : (xq + 1) * 16, q::2]
                            nc.tensor.matmul(
                                pt[q * 64 : (q + 1) * 64,
                                   sub * 512 : (sub + 1) * 512],
                                fin_t[g * 32 : g * 32 + 16, :],
                                rhs,
                                start=True, stop=True,
                                tile_position=(g * 32, q * 64),
                            )
                    evict(Xv[:, bl, xh * (TCOL // 32) : (xh + 1) * (TCOL // 32), :], pt[:], bin_t[:, 0:1])

            # ---------------- levels 1..5 (q-scheme) ----------------
            cur = X          # slab with free = (node, bl, h, w2)
            cur_nodes = 1
            tags = ["s1", "s0", "s1", "s0", "s1"]
            for lvl in range(1, 6):
                n2 = LVL_NODES[lvl - 1]
                grid = int(np.sqrt(n2))
                Hin = LVL_HIN[lvl - 1]
                W2in = Hin // 2
                Ho, W2o = Hin // 2, W2in // 2
                ncols_out = BG * Ho * W2o
                nxt = apool.tile([128, n2 * ncols_out], BF16,
                                 tag=tags[lvl - 1], name=f"a{lvl}_{phx}")
                curv = cur[:].rearrange("p (n b h w) -> p n b h w",
                                        n=cur_nodes, b=BG, h=Hin)
                nxtv = nxt[:].rearrange("p (n b h w) -> p n b h w",
                                        n=n2, b=BG, h=Ho)
                pgrid = int(np.sqrt(cur_nodes))
                if lvl <= KLVL:
                    # sibling-pair scheme: M=128=(coA,coB), shared parent rhs
                    Wo = W2in          # output width = rhs w-count
                    npairs = n2 // 2
                    # block = (bper b, hper h, all Wo) == 1024 cols (2 banks)
                    hper = min(Ho, TCOL // Wo)
                    bper = min(BG, max(1, TCOL // (Wo * hper)))
                    ncol = bper * hper * Wo
                    PGRP = 4           # pairs per weight DMA chunk
                    for g0 in range(0, npairs, PGRP):
                        gn = min(PGRP, npairs - g0)
                        wlt = wpool.tile([128, PGRP * 256], BF16, tag="wch",
                                         name=f"w{lvl}_{phx}_{g0}")
                        nc.sync.dma_start(
                            wlt[:, : gn * 256],
                            p[f"w{lvl}"][:, g0 * 256 : (g0 + gn) * 256],
                        )
                        for pr in range(g0, g0 + gn):
                            u, t = pr // (grid // 2), pr % (grid // 2)
                            nA = u * grid + 2 * t
                            nB = nA + 1
                            lp_ = pr - g0
                            pnode = (u // 2) * pgrid + t
                            # sub-splitting along b (or h) into 512-col chunks
                            nsub = ncol // 512
                            if bper >= nsub:
                                sb, sh = bper // nsub, hper
                            else:
                                sb, sh = 1, hper // (nsub // max(1, bper))
                            hsubs = hper // sh
                            for bs in range(0, BG, bper):
                                for h0 in range(0, Ho, hper):
                                    pt = ppool.tile(
                                        [128, ncol], F32, tag="ps",
                                        padded_shape=[128, TCOL],
                                        name=f"p{lvl}_{phx}_{pr}_{bs}_{h0}")
                                    for sub in range(nsub):
                                        b1 = bs + (sub // hsubs) * sb
                                        h1 = h0 + (sub % hsubs) * sh
                                        for x in (0, 1):
                                            rhs = curv[:, pnode, b1 : b1 + sb,
                                                       2 * h1 + x : 2 * (h1 + sh) : 2,
                                                       :]
                                            nc.tensor.matmul(
                                                pt[:, sub * 512 : (sub + 1) * 512],
                                                wlt[:, lp_ * 256 + x * 128 :
                                                    lp_ * 256 + (x + 1) * 128],
                                                rhs,
                                                start=(x == 0), stop=(x == 1),
                                            )
                                    for shalf, node in ((0, nA), (1, nB)):
                                        ptv = pt[shalf * 64 : (shalf + 1) * 64, :] \
                                            .rearrange("c (b h w) -> c b h w",
                                                       b=bper, h=hper)
                                        bias_ap = bias_t[lvl][
                                            shalf * 64 : (shalf + 1) * 64,
                                            node : node + 1]
                                        for par in (0, 1):
                                            evict(
                                                nxtv[par * 64 : (par + 1) * 64,
                                                     node, bs : bs + bper,
                                                     h0 : h0 + hper, :],
                                                ptv[:, :, :, par::2],
                                                bias_ap,
                                            )
                else:
                    # q-scheme (deep levels)
                    nblk = max(1, ncols_out // 512)
                    bper = BG // nblk
                    ncol = bper * Ho * W2o
                    for g0 in range(0, n2, WGRP):
                        gn = min(WGRP, n2 - g0)
                        wlt = wpool.tile([128, WGRP * 128], BF16, tag="wch",
                                         name=f"w{lvl}_{phx}_{g0}")
                        nc.sync.dma_start(
                            wlt[:, : gn * 128],
                            p[f"w{lvl}"][:, g0 * 128 : (g0 + gn) * 128],
                        )
                        for node in range(g0, g0 + gn):
                            ln = node - g0
                            pnode = node
                            for blk in range(nblk):
                                bs = blk * bper
                                pt = ppool.tile([128, ncol], F32, tag="ps",
                                                padded_shape=[128, TCOL],
                                                name=f"p{lvl}_{phx}_{node}_{blk}")
                                qx = [(x, q) for x in (0, 1) for q in (0, 1)] \
                                    if xouter else \
                                    [(x, q) for q in (0, 1) for x in (0, 1)]
                                for x, q in qx:
                                    rhs = curv[:, pnode, bs : bs + bper, x::2, q::2]
                                    nc.tensor.matmul(
                                        pt[q * 64 : (q + 1) * 64, :],
                                        wlt[:, ln * 128 + x * 64 :
                                            ln * 128 + (x + 1) * 64],
                                        rhs,
                                        start=(x == 0), stop=(x == 1),
                                        skip_group_check=xouter,
                                        tile_position=(0, q * 64),
                                    )
                                evict(
                                    nxtv[:, node, bs : bs + bper, :, :],
                                    pt[:],
                                    bias_t[lvl][:, node : node + 1],
                                )
                cur = nxt
                cur_nodes = n2

            # ---------------- level 6 (node pairs, M=64) ----------------
            # cur: [128, (n=64, bl, h=2, w2=1)] ; feats F [64=c, (node, bl)]
            F = fpool.tile([64, NK * NK * BG], BF16, tag="feats", name=f"f{phx}")
            Fv = F[:].rearrange("c (n b) -> c n b", n=NK * NK)
            curv = cur[:].rearrange("p (n b h w) -> p n b h w", n=64, b=BG, h=2)
            for g0 in range(0, 64, WGRP):
                w6t = wpool.tile([128, WGRP * 128], BF16, tag="wch",
                                 name=f"w6_{phx}_{g0}")
                nc.sync.dma_start(
                    w6t[:], p["w6"][:, g0 * 128 : (g0 + WGRP) * 128]
                )
                for pr in range(g0 // 2, (g0 + WGRP) // 2):
                    nA, nB = 2 * pr, 2 * pr + 1
                    pt = spool.tile([128, BG], F32, tag="ps", padded_shape=[128, TCOL],
                                    name=f"p6_{phx}_{pr}")
                    hx = [(h_, x_) for x_ in (0, 1) for h_ in (0, 1)] \
                        if xouter else \
                        [(h_, x_) for h_ in (0, 1) for x_ in (0, 1)]
                    for half, x in hx:
                        node = nA if half == 0 else nB
                        ln = node - g0
                        rhs = curv[:, node, :, x, 0]
                        nc.tensor.matmul(
                            pt[half * 64 : (half + 1) * 64, :],
                            w6t[:, ln * 128 + x * 64 :
                                ln * 128 + (x + 1) * 64],
                            rhs,
                            start=(x == 0), stop=(x == 1),
                            skip_group_check=xouter,
                            tile_position=(0, half * 64),
                        )
                    bias_ap = bias_t[6][:, pr : pr + 1]
                    evict_ctr[0] += 1
                    if evict_ctr[0] % 2 == 0:
                        nc.scalar.activation(Fv[0:64, nA, :], pt[0:64, :], AF.Relu,
                                             bias=bias_ap[0:64, :])
                        nc.scalar.activation(Fv[0:64, nB, :], pt[64:128, :], AF.Relu,
                                             bias=bias_ap[64:128, :])
                    else:
                        nc.vector.tensor_scalar(Fv[0:64, nA, :], pt[0:64, :],
                                                bias_ap[0:64, :], 0.0,
                                                op0=ALU.add, op1=ALU.max)
                        nc.vector.tensor_scalar(Fv[0:64, nB, :], pt[64:128, :],
                                                bias_ap[64:128, :], 0.0,
                                                op0=ALU.add, op1=ALU.max)

            # ---------------- dense ----------------
            t2s = fpool.tile([128, NK * NK * BG], F32, tag="t2s", name=f"t2s{phx}")
            t2sv = t2s[:].rearrange("m (n b) -> m n b", n=NK * NK)
            for g0 in range(0, 64, WGRP):
                wdt = wpool.tile([64, WGRP * 128], BF16, tag="wdch",
                                 name=f"wd_{phx}_{g0}")
                nc.sync.dma_start(
                    wdt[:], p["wd"][:, g0 * 128 : (g0 + WGRP) * 128]
                )
                for node in range(g0, g0 + WGRP):
                    ln = node - g0
                    pt = spool.tile([128, BG], F32, tag="ps", padded_shape=[128, TCOL],
                                    name=f"pd_{phx}_{node}")
                    nc.tensor.matmul(
                        pt[:],
                        wdt[:, ln * 128 : (ln + 1) * 128],
                        Fv[:, node, :],
                        start=True, stop=True,
                    )
                    evict_ctr[0] += 1
                    if evict_ctr[0] % 2 == 0:
                        nc.scalar.copy(t2sv[:, node, :], pt[:])
                    else:
                        nc.vector.tensor_copy(t2sv[:, node, :], pt[:])
            nc.sync.dma_start(
                t2[:, ph * NK * NK * BG : (ph + 1) * NK * NK * BG], t2s[:]
            )
    nc.compile()
    return nc


# ----------------------------------------------------------------------------
# entry point
# ----------------------------------------------------------------------------

def kernel(**inputs):
    inputs = {k: np.asarray(v) for k, v in inputs.items()}
    wblobs = _prep_weights(inputs)
    nc = _build_kernel()
    in_maps = []
    for c in range(NCORES):
        m = dict(wblobs)
        m["a0"] = _prep_input(inputs["in_data"][c * BC : (c + 1) * BC])
        in_maps.append(m)
    res = run_bass_kernel_spmd(nc, in_maps, list(range(NCORES)))
    outs = [_decode_output(res.results[c]["t2"]) for c in range(NCORES)]
    return np.concatenate(outs, axis=0).astype(np.float32)


if __name__ == "__main__":
    import reference as ref

    inputs = {k: np.asarray(v) for k, v in ref.setup_inputs().items()}
    expected = np.asarray(ref.reference(**inputs))
    actual = kernel(**inputs)
    err = np.abs(actual - expected).max()
    rel = err / np.abs(expected).max()
    print("absmax:", err, "rel:", rel)



# revision 14
# speedup vs baseline: 1.4915x; 1.4915x over previous
"""Trainium2 Bass kernel for nn_ButterflyLayer2D (butterfly 2D CNN).

Strategy: pure data parallel over 8 NeuronCores (16 batch each).

Layout invariant: activations live in SBUF as [128 = (w%2)*64 + c,
(node, b, h, w//2)] bf16.  Each per-node 2x2-stride-2 conv is then 4
matmuls per 512-col psum tile: contraction K=128=(dw,c), accumulated
over x=dh (start/stop), with output w-parity q selected by slicing the
rhs w2 dim (q::2) and writing psum rows q*64+c via col tile_position
(0, q*64).  The psum tile [128=(q,c_out), 512] is exactly the next
level's layout, so every eviction is a contiguous full-width
[128, 512] relu+bias op alternating between ScalarE and VectorE.

Tail levels pack several nodes per psum tile (4 at L4, 16 at L5, all
64 at L6, 32 per tile in the dense) so evictions stay [128, 512].
When a level's bias is nonzero (bias must be per-partition-constant
within one evict) those levels fall back to per-node evictions.

Weights stream from HBM in 16-node chunks through an 8-deep ring,
giving ~2 MB of prefetch depth.  Input conv emission is interleaved
with L1 (lag 2 batch-elements) because the input phase is
eviction-bound and would otherwise idle the PE into HAM re-throttle.
"""

import numpy as np
from contextlib import ExitStack

import concourse.bass as bass
import concourse.tile as tile
from concourse import bacc, mybir
from concourse.bass_utils import run_bass_kernel_spmd

F32 = mybir.dt.float32
BF16 = mybir.dt.bfloat16
AF = mybir.ActivationFunctionType
ALU = mybir.AluOpType

B, IN, NLVL, KLVL, C = 128, 256, 6, 3, 64
NK, OU, OV = 8, 8, 8
NCORES = 8
BC = B // NCORES          # 16 per-core batch
BG = BC                   # single phase
HALF = BG // 4            # input-conv b-subgroups per partition group
LVL_NODES = [4, 16, 64, 64, 64, 64]          # nodes per level
LVL_HIN = [64, 32, 16, 8, 4, 2]              # spatial H into each level
WCH = 16                  # weight streaming chunk (nodes)


# ----------------------------------------------------------------------------
# host-side pre-arrangement
# ----------------------------------------------------------------------------

def _prep_weights(inputs):
    """Weights/biases blobs shared by all cores."""
    import ml_dtypes

    out = {}
    zflags = {}
    # input filter: lhsT [16=(p,q), 64], replicated at partition bases 0/32/64/96
    fin = inputs["in_filter"][:, :, 0, :].reshape(16, C).astype(np.float32)
    finr = np.zeros((128, C), np.float32)
    for g in range(4):
        finr[g * 32 : g * 32 + 16] = fin
    out["fin"] = finr.astype(ml_dtypes.bfloat16)
    out["bin"] = np.concatenate([inputs["in_bias"], inputs["in_bias"]]).reshape(
        128, 1
    ).astype(np.float32)

    for lvl in range(1, NLVL + 1):
        f = inputs[f"f{lvl}"].astype(np.float32)  # [n,n,2,2,C,C] (x=dh,y=dw,ci,co)
        n = f.shape[0]
        assert n == 2 ** min(lvl, KLVL)
        # lhsT per node: [(dw*64+ci), (dh*64+co)]; blob [128, node*(2C)]
        w = f.transpose(0, 1, 3, 4, 2, 5).reshape(n * n, 2 * C, 2 * C)
        out[f"w{lvl}"] = np.ascontiguousarray(w.transpose(1, 0, 2)).reshape(
            128, n * n * 128
        ).astype(ml_dtypes.bfloat16)
        b = inputs[f"b{lvl}"].astype(np.float32).reshape(n * n, C)
        zflags[lvl] = not np.any(b)
        if lvl < NLVL:
            # [128, nodes]: rows (q,c) with bias duplicated across q
            bb = np.concatenate([b, b], axis=1)  # [nodes, 128]
            out[f"b{lvl}"] = np.ascontiguousarray(bb.T)
        else:
            # lvl6 node-pair scheme: psum rows = (cA, cB) for pair (2k, 2k+1)
            bb = b.reshape(n * n // 2, 2 * C)  # [pairs, (cA,cB)]
            out[f"b{lvl}"] = np.ascontiguousarray(bb.T)  # [128, 32]
    # dense, stacked pairs: rows 0:64 = c for even node, 64:128 = c for odd;
    # cols (pair, (r,ou,ov))
    wd = inputs["Wd"].astype(np.float32).reshape(NK * NK, 2, C, OU * OV)
    wdn = wd.transpose(0, 2, 1, 3).reshape(NK * NK, C, 2 * OU * OV)
    wds = np.zeros((128, (NK * NK // 2) * 2 * OU * OV), np.float32)
    for p in range(NK * NK // 2):
        wds[0:64, p * 128 : (p + 1) * 128] = wdn[2 * p]
        wds[64:128, p * 128 : (p + 1) * 128] = wdn[2 * p + 1]
    out["wd"] = np.ascontiguousarray(wds).astype(ml_dtypes.bfloat16)
    return out, zflags


def _prep_input(in_data_core):
    """Per-core input blob: [64 = (b%4)*16 + (i%4)*4 + (j%4),
    (b//4%2, x=i//4, y4=j//4)] packed (no zero rows)."""
    import ml_dtypes

    ind = in_data_core[:, :, :, 0]  # [16, 256, 256]
    a = ind.reshape(HALF, 4, 64, 4, 64, 4)      # [half, g, x, p, y4, q]
    a = a.transpose(1, 3, 5, 0, 2, 4)           # [g, p, q, half, x, y4]
    return np.ascontiguousarray(a).reshape(64, HALF * 64 * 64).astype(
        ml_dtypes.bfloat16
    )


def _decode_output(t2_core):
    """t2 [128=(r,ou,ov), (par, pair, bl)], node=2*pair+par -> [16, 64, 64, 2]."""
    t = t2_core.reshape(2, OU, OV, 2, 32, BG)       # r,ou,ov,par,pair,bl
    t = t.transpose(0, 1, 2, 4, 3, 5)               # r,ou,ov,pair,par,bl
    t = t.reshape(2, OU, OV, NK, NK, BG)            # r,ou,ov,u,v,bl
    t = t.transpose(5, 3, 1, 4, 2, 0)               # bl,u,ou,v,ov,r
    return np.ascontiguousarray(t).reshape(BG, NK * OU, NK * OV, 2)


# ----------------------------------------------------------------------------
# device kernel
# ----------------------------------------------------------------------------

def _build_kernel(zflags, debug=False):
    nc = bacc.Bacc(None, target_bir_lowering=False)
    p = {}
    p["a0"] = nc.declare_dram_parameter("a0", [64, HALF * 64 * 64], BF16, isOutput=False)
    p["fin"] = nc.declare_dram_parameter("fin", [128, C], BF16, isOutput=False)
    p["bin"] = nc.declare_dram_parameter("bin", [128, 1], F32, isOutput=False)
    for lvl in range(1, NLVL + 1):
        n2 = LVL_NODES[lvl - 1]
        p[f"w{lvl}"] = nc.declare_dram_parameter(f"w{lvl}", [128, n2 * 128], BF16, isOutput=False)
        bcols = n2 if lvl < NLVL else n2 // 2
        p[f"b{lvl}"] = nc.declare_dram_parameter(f"b{lvl}", [128, bcols], F32, isOutput=False)
    p["wd"] = nc.declare_dram_parameter("wd", [128, 32 * 128], BF16, isOutput=False)
    t2 = nc.declare_dram_parameter("t2", [128, NK * NK * BG], F32, isOutput=True)
    dbg = {}
    if debug:
        dbg["X"] = nc.declare_dram_parameter("dbgX", [128, BG * 64 * 32], BF16, isOutput=True)
        for lvl in range(1, 6):
            n2 = LVL_NODES[lvl - 1]
            Ho = LVL_HIN[lvl - 1] // 2
            dbg[lvl] = nc.declare_dram_parameter(
                f"dbgL{lvl}", [128, n2 * BG * Ho * (Ho // 2 if Ho > 1 else 1)], BF16, isOutput=True)
        dbg["F"] = nc.declare_dram_parameter("dbgF", [128, 32 * BG], BF16, isOutput=True)

    evict_ctr = [0]

    def evict(out_ap, psum_ap, bias_ap):
        """relu(psum + bias) -> sbuf, alternating engines to split the load."""
        evict_ctr[0] += 1
        if evict_ctr[0] % 2 == 0:
            if bias_ap is None:
                nc.scalar.activation(out_ap, psum_ap, AF.Relu)
            else:
                nc.scalar.activation(out_ap, psum_ap, AF.Relu, bias=bias_ap)
        else:
            if bias_ap is None:
                nc.vector.tensor_scalar(out_ap, psum_ap, 0.0, None, op0=ALU.max)
            else:
                nc.vector.tensor_scalar(out_ap, psum_ap, bias_ap, 0.0,
                                        op0=ALU.add, op1=ALU.max)

    with tile.TileContext(nc) as tc, ExitStack() as ctx:
        const = ctx.enter_context(tc.tile_pool(name="const", bufs=1))
        wpool = ctx.enter_context(tc.tile_pool(name="wts", bufs=8))
        apool = ctx.enter_context(tc.tile_pool(name="acts", bufs=1))
        inpool = ctx.enter_context(tc.tile_pool(name="inp", bufs=1))
        fpool = ctx.enter_context(tc.tile_pool(name="feat", bufs=1))
        ppool = ctx.enter_context(tc.tile_pool(name="ps", bufs=8, space="PSUM"))

        # constants: input filter, biases (all small, loaded once)
        fin_t = const.tile([128, C], BF16)
        nc.sync.dma_start(fin_t[:], p["fin"][:])
        bin_t = const.tile([128, 1], F32)
        nc.sync.dma_start(bin_t[:], p["bin"][:])
        bias_t = {}
        for lvl in range(1, NLVL + 1):
            bcols = LVL_NODES[lvl - 1] if lvl < NLVL else LVL_NODES[lvl - 1] // 2
            bias_t[lvl] = const.tile([128, bcols], F32, tag=f"bias{lvl}", name=f"bias{lvl}")
            nc.sync.dma_start(bias_t[lvl][:], p[f"b{lvl}"][:])

        # ------------- input staging: 8 chunks, (half, g) -------------
        a0s = inpool.tile([128, HALF * 64 * 64], BF16, tag="a0s", name="a0s")
        qcols = HALF * 64 * 64
        for half in range(2):
            for g in range(4):
                nc.sync.dma_start(
                    a0s[g * 32 : g * 32 + 16,
                        half * (qcols // 2) : (half + 1) * (qcols // 2)],
                    p["a0"][g * 16 : (g + 1) * 16,
                            half * (qcols // 2) : (half + 1) * (qcols // 2)],
                )
        a0v = a0s[:].rearrange("p (h x y) -> p h x y", h=HALF, x=64)

        def wchunk(lvl, g0, gn, rows=128):
            """Stream a 16-node chunk of level weights into the ring."""
            wlt = wpool.tile([128, WCH * 128], BF16, tag="wch",
                             name=f"w{lvl}_{g0}")
            src = p[f"w{lvl}"] if lvl != "d" else p["wd"]
            nc.sync.dma_start(
                wlt[:rows, : gn * 128],
                src[:rows, g0 * 128 : (g0 + gn) * 128],
            )
            return wlt

        # ---------------- input conv + L1, interleaved ----------------
        # X slab: [128=(w%2,c), (bl, h=64, w2=32)]
        X = apool.tile([128, BG * 64 * 32], BF16, tag="sA", name="x0")
        Xv = X[:].rearrange("p (b h w) -> p b h w", b=BG, h=64)
        L1n = LVL_NODES[0]
        L1out = apool.tile([128, L1n * BG * 32 * 16], BF16, tag="sB", name="a1")
        L1v = L1out[:].rearrange("p (n b h w) -> p n b h w", n=L1n, b=BG, h=32)
        w1t = wchunk(1, 0, L1n)

        def input_bl(bl):
            g, half = bl % 4, bl // 4
            for xq in range(4):
                pt = ppool.tile([128, 512], F32, tag="ps",
                                padded_shape=[128, 512], name=f"pin{bl}_{xq}")
                for q in (0, 1):
                    rhs = a0v[g * 32 : g * 32 + 16, half,
                              xq * 16 : (xq + 1) * 16, q::2]
                    nc.tensor.matmul(
                        pt[q * 64 : (q + 1) * 64, :],
                        fin_t[g * 32 : g * 32 + 16, :],
                        rhs,
                        start=True, stop=True,
                        tile_position=(g * 32, q * 64),
                    )
                evict(Xv[:, bl, xq * 16 : (xq + 1) * 16, :], pt[:], bin_t[:, 0:1])

        def l1_bl(bl):
            for n in range(L1n):
                pt = ppool.tile([128, 512], F32, tag="ps",
                                padded_shape=[128, 512], name=f"p1_{bl}_{n}")
                for x in (0, 1):
                    for q in (0, 1):
                        rhs = Xv[:, bl, x::2, q::2]
                        nc.tensor.matmul(
                            pt[q * 64 : (q + 1) * 64, :],
                            w1t[:, n * 128 + x * 64 : n * 128 + (x + 1) * 64],
                            rhs,
                            start=(x == 0), stop=(x == 1),
                            skip_group_check=True,
                            tile_position=(0, q * 64),
                        )
                evict(L1v[:, n, bl, :, :], pt[:], bias_t[1][:, n : n + 1])

        for bl in range(BG):
            input_bl(bl)
            if bl >= 2:
                l1_bl(bl - 2)
        l1_bl(BG - 2)
        l1_bl(BG - 1)
        if debug:
            nc.sync.dma_start(dbg["X"][:], X[:])
            nc.sync.dma_start(dbg[1][:], L1out[:])

        # ---------------- levels 2..5 (q-scheme) ----------------
        cur, cur_nodes = L1out, L1n
        tags = {2: "sA", 3: "sB", 4: "sA", 5: "sB"}
        for lvl in range(2, 6):
            n2 = LVL_NODES[lvl - 1]
            grid = int(np.sqrt(n2))
            Hin = LVL_HIN[lvl - 1]
            Ho, W2o = Hin // 2, Hin // 4
            pcols = BG * Ho * W2o               # output cols per node
            npt = max(1, 512 // pcols)          # nodes per psum tile
            nbs = max(1, pcols // 512)          # b-chunks per node
            bper = BG // nbs
            csz = min(pcols, 512)               # psum cols per node per tile
            zb = zflags[lvl]
            nxt = apool.tile([128, n2 * BG * Ho * W2o], BF16,
                             tag=tags[lvl], name=f"a{lvl}")
            curv = cur[:].rearrange("p (n b h w) -> p n b h w",
                                    n=cur_nodes, b=BG, h=Hin)
            nxtv = nxt[:].rearrange("p (n b h w) -> p n b h w",
                                    n=n2, b=BG, h=Ho)
            pgrid = int(np.sqrt(cur_nodes))
            for g0 in range(0, n2, WCH):
                wlt = wchunk(lvl, g0, min(WCH, n2 - g0))
                for t0 in range(g0, g0 + min(WCH, n2 - g0), npt):
                    for bs in range(nbs):
                        pt = ppool.tile([128, npt * csz], F32, tag="ps",
                                        padded_shape=[128, 512],
                                        name=f"p{lvl}_{t0}_{bs}")
                        for n in range(t0, t0 + npt):
                            if lvl <= KLVL:
                                u, v = n // grid, n % grid
                                pn = (u // 2) * pgrid + (v // 2)
                            else:
                                pn = n
                            ln = n - g0
                            lt = n - t0
                            for x in (0, 1):
                                for q in (0, 1):
                                    rhs = curv[:, pn,
                                               bs * bper : (bs + 1) * bper,
                                               x::2, q::2]
                                    nc.tensor.matmul(
                                        pt[q * 64 : (q + 1) * 64,
                                           lt * csz : (lt + 1) * csz],
                                        wlt[:, ln * 128 + x * 64 :
                                            ln * 128 + (x + 1) * 64],
                                        rhs,
                                        start=(x == 0), stop=(x == 1),
                                        skip_group_check=True,
                                        tile_position=(0, q * 64),
                                    )
                        if npt == 1 or not zb:
                            for n in range(t0, t0 + npt):
                                lt = n - t0
                                evict(
                                    nxtv[:, n, bs * bper : (bs + 1) * bper, :, :],
                                    pt[:, lt * csz : (lt + 1) * csz],
                                    bias_t[lvl][:, n : n + 1],
                                )
                        else:
                            evict(
                                nxtv[:, t0 : t0 + npt, :, :, :],
                                pt[:],
                                None,
                            )
            if debug:
                nc.sync.dma_start(dbg[lvl][:], nxt[:])
            cur, cur_nodes = nxt, n2

        # ---------------- level 6 (node pairs, M=64) ----------------
        # cur: [128, (n=64, bl, h=2, w2=1)] ; feats F [128=(cEven,cOdd), (pair, b)]
        F = fpool.tile([128, 32 * BG], BF16, tag="feats", name="feats")
        Fv = F[:].rearrange("p (pr b) -> p pr b", pr=32)
        curv = cur[:].rearrange("p (n b h w) -> p n b h w", n=64, b=BG, h=2)
        zb6 = zflags[6]
        pt6 = ppool.tile([128, 512], F32, tag="ps",
                         padded_shape=[128, 512], name="p6")
        for g0 in range(0, 64, WCH):
            w6t = wchunk(6, g0, WCH)
            for pr in range(g0 // 2, (g0 + WCH) // 2):
                for half in (0, 1):
                    node = 2 * pr + half
                    ln = node - g0
                    for x in (0, 1):
                        rhs = curv[:, node, :, x, 0]
                        nc.tensor.matmul(
                            pt6[half * 64 : (half + 1) * 64,
                                pr * BG : (pr + 1) * BG],
                            w6t[:, ln * 128 + x * 64 : ln * 128 + (x + 1) * 64],
                            rhs,
                            start=(x == 0), stop=(x == 1),
                            skip_group_check=True,
                            tile_position=(0, half * 64),
                        )
        if zb6:
            evict(F[:], pt6[:], None)
        else:
            for pr in range(32):
                evict(Fv[:, pr, :], pt6[:, pr * BG : (pr + 1) * BG],
                      bias_t[6][:, pr : pr + 1])

        if debug:
            nc.sync.dma_start(dbg["F"][:], F[:])

        # ---------------- dense ----------------
        # t2 cols are parity-major: (par, pair, b); node = 2*pair + par.
        # One psum tile per (chunk, parity) so each tile only ever receives
        # matmuls from a single PE row-group.
        t2s = fpool.tile([128, NK * NK * BG], F32, tag="t2s", name="t2s")
        for t in range(2):
            wdt = wchunk("d", t * 16, 16)
            for par in (0, 1):
                ptd = ppool.tile([128, 16 * BG], F32, tag="ps",
                                 padded_shape=[128, 512], name=f"pd{t}_{par}")
                for lp in range(16):
                    p_ = t * 16 + lp
                    nc.tensor.matmul(
                        ptd[:, lp * BG : (lp + 1) * BG],
                        wdt[par * 64 : (par + 1) * 64,
                            lp * 128 : (lp + 1) * 128],
                        Fv[par * 64 : (par + 1) * 64, p_, :],
                        start=True, stop=True,
                        tile_position=(par * 64, 0),
                    )
                evict_ctr[0] += 1
                dst = t2s[:, par * 512 + t * 256 : par * 512 + (t + 1) * 256]
                if evict_ctr[0] % 2 == 0:
                    nc.scalar.copy(dst, ptd[:])
                else:
                    nc.vector.tensor_copy(dst, ptd[:])
                nc.sync.dma_start(
                    t2[:, par * 512 + t * 256 : par * 512 + (t + 1) * 256], dst
                )
    nc.compile()
    return nc


# ----------------------------------------------------------------------------
# entry point
# ----------------------------------------------------------------------------

def kernel(**inputs):
    inputs = {k: np.asarray(v) for k, v in inputs.items()}
    wblobs, zflags = _prep_weights(inputs)
    nc = _build_kernel(zflags)
    in_maps = []
    for c in range(NCORES):
        m = dict(wblobs)
        m["a0"] = _prep_input(inputs["in_data"][c * BC : (c + 1) * BC])
        in_maps.append(m)
    res = run_bass_kernel_spmd(nc, in_maps, list(range(NCORES)))
    outs = [_decode_output(res.results[c]["t2"]) for c in range(NCORES)]
    return np.concatenate(outs, axis=0).astype(np.float32)


if __name__ == "__main__":
    import reference as ref

    inputs = {k: np.asarray(v) for k, v in ref.setup_inputs().items()}
    expected = np.asarray(ref.reference(**inputs))
    actual = kernel(**inputs)
    err = np.abs(actual - expected).max()
    rel = err / np.abs(expected).max()
    print("absmax:", err, "rel:", rel)
